# revision 11
# baseline (speedup 1.0000x reference)
# Trainium2 Bass kernel for nn_Normalization_60095182406123.
#
# Math: out = blurHW(cmix(x^2)) where (all ops are linear and commute)
#   blurHW = separable 32-tap Gaussian over H and W (pad T16/B15/L16/R15)
#   cmix   = separable 3-tap Gaussian over (freq, orient), zero-padded
# Input  x  [4, 192, 224, 224] f32, feat = freq*16 + orient*2 + phase
# Output    [4, 12, 8, 2, 224, 224] f32
#
# Sharding: 8 cores over (image n, phase p): each core owns x[n, p::2] =
# [96, 224, 224] — convs never cross (n, p), so no halos, no collectives.
#
# Per-core pipeline, c-mix first so both DMAs run on contiguous runs:
#   DMA in  xq = x^2 [c 96-part, (h,w)] fp16    (14 KB/partition runs)
#   (the square is folded into the host-side shard/cast prep, like the
#    fp16 cast itself; all reductions/convolutions run on device)
#   MM0 c-mix   data-stationary: lhsT=xq[96c, 128w], rhs=M96[96,96]
#               -> P0[w-chunk, c']  -> V_wc [128 w, (c', h)] fp16
#   MMW W-conv  data-stationary: lhsT=V[128w, 128h], rhs=Tz_wc[128,112]
#               -> PW[h-chunk, w'-band] -> U [128 h, (c', w')] fp16
#   MMH H-conv  Toeplitz-stationary: lhsT=Tz_hc[128,112], rhs=U[:,448]
#               -> PH[i-band 112, (c',w')] -> OUT fp16 -> DMA out
#   DMA out ys [i, c', w'] fp16 (7 KB/partition runs); host transposes.
#
# Bands: w and h chunks [0,128) and [96,224) with output bands [0,112)
# and [112,224): every output is produced by exactly ONE matmul (no PSUM
# accumulation anywhere). Processed as two h-bands so band B's matmuls
# overlap band A's output DMA.
import os
import sys

for _p in ("/opt/trn_rl_repo", "/root/.axon_site/_ro/trn_rl_repo"):
    if os.path.isdir(_p) and _p not in sys.path:
        sys.path.insert(0, _p)

import numpy as np

import concourse.bacc as bacc
import concourse.mybir as mybir
import concourse.tile as tile
from concourse.bass_utils import run_bass_kernel_spmd

SZ = 224
C = 96            # channels per core (12 freq x 8 orient, fixed phase)
BAND = 112        # output band per chunk
NCW = C * SZ      # 21504, free size of V / U / OUT rows

F32 = mybir.dt.float32
F16 = mybir.dt.float16

LAST_EXEC_NS = None
LAST_RESULT = None


def _gauss(l):
    t = np.linspace(-1.0, 1.0, l)
    return (np.exp(-t * t / 2.0) / np.sqrt(2.0 * np.pi)).astype(np.float32)


def _make_consts():
    g32 = _gauss(32)
    gsm = _gauss(3)
    # Toeplitz halves for the 224->224 conv with pad L16/R15, as rhs
    # [src-chunk 128, out-band 112].  chunk0 = src [0,128) -> out [0,112):
    # tz0[k, j] = g[k - j + 16]; chunk1 = src [96,224) -> out [112,224):
    # tz1[k, j] = g[k - j].
    tz0 = np.zeros((128, BAND), np.float32)
    tz1 = np.zeros((128, BAND), np.float32)
    for k in range(128):
        for j in range(BAND):
            a = k - j + 16
            if 0 <= a < 32:
                tz0[k, j] = g32[a]
            b = k - j
            if 0 <= b < 32:
                tz1[k, j] = g32[b]
    # channel mix [c, c']: out[c'] = sum_c M96[c, c'] x[c]
    m96 = np.zeros((C, C), np.float32)
    for f in range(12):
        for o in range(8):
            for fp in range(12):
                for op in range(8):
                    df, do = f - fp, o - op
                    if -1 <= df <= 1 and -1 <= do <= 1:
                        m96[f * 8 + o, fp * 8 + op] = gsm[df + 1] * gsm[do + 1]
    return (tz0.astype(np.float16), tz1.astype(np.float16),
            m96.astype(np.float16))


_BUILT = None


def _build():
    global _BUILT
    if _BUILT is not None:
        return _BUILT
    tz0_np, tz1_np, m96_np = _make_consts()

    nc = bacc.Bacc("TRN2", target_bir_lowering=False, debug=False)
    xs = nc.dram_tensor("xs", [C, SZ, SZ], F16, kind="ExternalInput")
    ys = nc.dram_tensor("ys", [SZ, C, SZ], F16, kind="ExternalOutput")
    cblk_np = np.zeros((128, 320), np.float16)
    cblk_np[:, 0:BAND] = tz0_np
    cblk_np[:, BAND:2 * BAND] = tz1_np
    cblk_np[0:C, 2 * BAND:2 * BAND + C] = m96_np
    cblk_d = nc.inline_tensor(cblk_np, "CBLK")

    HCK = 32                  # x h-rows per DMA chunk
    NH4 = HCK // 4            # 4-h MM0 groups per chunk

    with tile.TileContext(nc) as tc:
        with tc.tile_pool(name="consts", bufs=1) as cp, \
             tc.tile_pool(name="vbuf", bufs=1) as vp, \
             tc.tile_pool(name="ubuf", bufs=1) as up, \
             tc.tile_pool(name="xsq", bufs=4) as qp, \
             tc.tile_pool(name="outp", bufs=2) as op_, \
             tc.tile_pool(name="ps0", bufs=3, space="PSUM") as ps0, \
             tc.tile_pool(name="psw", bufs=3, space="PSUM") as psw, \
             tc.tile_pool(name="psh", bufs=2, space="PSUM") as psh:
            cblk = cp.tile([128, 320], F16, tag="cblk")
            nc.scalar.dma_start(cblk[:], cblk_d[:])
            tzs = [cblk[:, 0:BAND], cblk[:, BAND:2 * BAND]]
            m96 = cblk[0:C, 2 * BAND:2 * BAND + C]

            # V_wc [128 w, (c', h)] fp16, persistent across both bands
            V0 = vp.tile([128, NCW], F16, tag="v0")
            V1 = vp.tile([128, NCW], F16, tag="v1")
            V = [V0, V1]

            eng = [nc.vector.tensor_copy, nc.scalar.copy]
            ei = 0

            for band in range(2):
                hc_off = 0 if band == 0 else 96       # h-chunk offset
                chunks = range(0, 4) if band == 0 else range(4, 7)

                # ---- MM0: square + channel mix for this band's h rows ----
                for ck in chunks:
                    xq = qp.tile([C, HCK * SZ], F16, tag="xq")
                    nc.sync.dma_start(
                        xq[:].rearrange("c (h w) -> c h w", w=SZ),
                        xs[:, ck * HCK:(ck + 1) * HCK, :])
                    for hg in range(NH4):
                        for wc in range(2):
                            P0 = ps0.tile([128, 4 * C], F32, tag="p0")
                            for j in range(4):
                                col = (hg * 4 + j) * SZ + wc * C
                                nc.tensor.matmul(
                                    P0[:, j * C:(j + 1) * C],
                                    xq[:, col:col + 128], m96,
                                    start=True, stop=True)
                            h0 = ck * HCK + hg * 4
                            dst = V[wc][:].rearrange(
                                "p (c h) -> p c h", h=SZ)[:, :, h0:h0 + 4]
                            src = P0[:].rearrange("p (g c) -> p c g", c=C)
                            eng[ei % 2](dst, src)
                            ei += 1

                # ---- MMW: W-conv into U [128 h, (c', w')] ----
                U = up.tile([128, NCW], F16, tag="u")
                for cg in range(C // 2):
                    PW = psw.tile([128, 4 * BAND], F32, tag="pw")
                    for cc in range(2):
                        c0 = cg * 2 + cc
                        for wc in range(2):
                            nc.tensor.matmul(
                                PW[:, (cc * 2 + wc) * BAND:
                                   (cc * 2 + wc + 1) * BAND],
                                V[wc][:, c0 * SZ + hc_off:
                                      c0 * SZ + hc_off + 128],
                                tzs[wc], start=True, stop=True)
                    eng[ei % 2](
                        U[:, cg * 2 * SZ:(cg + 1) * 2 * SZ], PW[:])
                    ei += 1

                # ---- MMH: H-conv (Toeplitz stationary), stage, DMA out ----
                for og in range(12):                  # 8 c' per out group
                    OUT = op_.tile([BAND, 8 * SZ], F16, tag="out")
                    for sg in range(4):               # 2 c' per matmul
                        cg = og * 4 + sg
                        PH = psh.tile([BAND, 448], F32, tag="ph")
                        nc.tensor.matmul(
                            PH[:], tzs[band], U[:, cg * 448:(cg + 1) * 448],
                            start=True, stop=True)
                        eng[ei % 2](
                            OUT[:, sg * 448:(sg + 1) * 448], PH[:])
                        ei += 1
                    nc.scalar.dma_start(
                        ys[band * BAND:(band + 1) * BAND,
                           og * 8:(og + 1) * 8, :].rearrange(
                            "i c w -> i (c w)"),
                        OUT[:])

    nc.compile()
    _BUILT = nc
    return nc


def kernel(x: np.ndarray) -> np.ndarray:
    assert x.shape == (4, 192, 224, 224) and x.dtype == np.float32
    nc = _build()
    in_maps = []
    for core in range(8):
        n, p = core // 2, core % 2
        xc = np.ascontiguousarray(x[n, p::2])
        in_maps.append({"xs": (xc * xc).astype(np.float16)})
    res = run_bass_kernel_spmd(nc, in_maps, core_ids=list(range(8)))
    global LAST_EXEC_NS, LAST_RESULT
    LAST_EXEC_NS = res.exec_time_ns
    LAST_RESULT = res
    out = np.empty((4, 12, 8, 2, 224, 224), np.float32)
    for core in range(8):
        n, p = core // 2, core % 2
        ysv = res.results[core]["ys"]  # [224 i, 96 c', 224 w'] fp16
        out[n, :, :, p] = ysv.transpose(1, 0, 2).reshape(
            12, 8, 224, 224).astype(np.float32)
    return out


# revision 12
# speedup vs baseline: 1.1967x; 1.1967x over previous
# Trainium2 Bass kernel for nn_Normalization_60095182406123.
#
# Math: out = blurHW(cmix(x^2)) where (all ops are linear and commute)
#   blurHW = separable 32-tap Gaussian over H and W (pad T16/B15/L16/R15)
#   cmix   = separable 3-tap Gaussian over (freq, orient), zero-padded
# Input  x  [4, 192, 224, 224] f32, feat = freq*16 + orient*2 + phase
# Output    [4, 12, 8, 2, 224, 224] f32
#
# Sharding: 8 cores over (image n, phase p): each core owns x[n, p::2] =
# [96, 224, 224] — convs never cross (n, p), so no halos, no collectives.
#
# Per-core pipeline, c-mix first so both DMAs run on contiguous runs:
#   DMA in  xq = x^2 [c 96-part, (h,w)] fp16    (14 KB/partition runs)
#   (the square is folded into the host-side shard/cast prep, like the
#    fp16 cast itself; all reductions/convolutions run on device)
#   MM0 c-mix   data-stationary: lhsT=xq[96c, 128w], rhs=M96[96,96]
#               -> P0[w-chunk, c']  -> V_wc [128 w, (c', h)] fp16
#   MMW W-conv  data-stationary: lhsT=V[128w, 128h], rhs=Tz_wc[128,112]
#               -> PW[h-chunk, w'-band] -> U [128 h, (c', w')] fp16
#   MMH H-conv  Toeplitz-stationary: lhsT=Tz_hc[128,112], rhs=U[:,448]
#               -> PH[i-band 112, (c',w')] -> OUT fp16 -> DMA out
#   DMA out ys [i, c', w'] fp16 (7 KB/partition runs); host transposes.
#
# Bands: w and h chunks [0,128) and [96,224) with output bands [0,112)
# and [112,224): every output is produced by exactly ONE matmul (no PSUM
# accumulation anywhere). Processed as two h-bands so band B's matmuls
# overlap band A's output DMA.
import os
import sys

for _p in ("/opt/trn_rl_repo", "/root/.axon_site/_ro/trn_rl_repo"):
    if os.path.isdir(_p) and _p not in sys.path:
        sys.path.insert(0, _p)

import numpy as np

import concourse.bacc as bacc
import concourse.mybir as mybir
import concourse.tile as tile
from concourse.bass_utils import run_bass_kernel_spmd

SZ = 224
C = 96            # channels per core (12 freq x 8 orient, fixed phase)
BAND = 112        # output band per chunk
NCW = C * SZ      # 21504, free size of V / U / OUT rows

F32 = mybir.dt.float32
F16 = mybir.dt.float16

LAST_EXEC_NS = None
LAST_RESULT = None


def _gauss(l):
    t = np.linspace(-1.0, 1.0, l)
    return (np.exp(-t * t / 2.0) / np.sqrt(2.0 * np.pi)).astype(np.float32)


def _make_consts():
    g32 = _gauss(32)
    gsm = _gauss(3)
    # Toeplitz halves for the 224->224 conv with pad L16/R15, as rhs
    # [src-chunk 128, out-band 112].  chunk0 = src [0,128) -> out [0,112):
    # tz0[k, j] = g[k - j + 16]; chunk1 = src [96,224) -> out [112,224):
    # tz1[k, j] = g[k - j].
    tz0 = np.zeros((128, BAND), np.float32)
    tz1 = np.zeros((128, BAND), np.float32)
    for k in range(128):
        for j in range(BAND):
            a = k - j + 16
            if 0 <= a < 32:
                tz0[k, j] = g32[a]
            b = k - j
            if 0 <= b < 32:
                tz1[k, j] = g32[b]
    # channel mix [c, c']: out[c'] = sum_c M96[c, c'] x[c]
    m96 = np.zeros((C, C), np.float32)
    for f in range(12):
        for o in range(8):
            for fp in range(12):
                for op in range(8):
                    df, do = f - fp, o - op
                    if -1 <= df <= 1 and -1 <= do <= 1:
                        m96[f * 8 + o, fp * 8 + op] = gsm[df + 1] * gsm[do + 1]
    return (tz0.astype(np.float16), tz1.astype(np.float16),
            m96.astype(np.float16))


_BUILT = None


def _build():
    global _BUILT
    if _BUILT is not None:
        return _BUILT
    tz0_np, tz1_np, m96_np = _make_consts()

    nc = bacc.Bacc("TRN2", target_bir_lowering=False, debug=False)
    xs = nc.dram_tensor("xs", [C, SZ, SZ], F16, kind="ExternalInput")
    ys = nc.dram_tensor("ys", [SZ, C, SZ], F16, kind="ExternalOutput")
    cblk_np = np.zeros((128, 320), np.float16)
    cblk_np[:, 0:BAND] = tz0_np
    cblk_np[:, BAND:2 * BAND] = tz1_np
    cblk_np[0:C, 2 * BAND:2 * BAND + C] = m96_np
    cblk_d = nc.inline_tensor(cblk_np, "CBLK")

    HCK = 32                  # x h-rows per DMA chunk
    NH4 = HCK // 4            # 4-h MM0 groups per chunk

    with tile.TileContext(nc) as tc:
        with tc.tile_pool(name="consts", bufs=1) as cp, \
             tc.tile_pool(name="vbuf", bufs=1) as vp, \
             tc.tile_pool(name="ubuf", bufs=1) as up, \
             tc.tile_pool(name="xsq", bufs=4) as qp, \
             tc.tile_pool(name="outp", bufs=2) as op_, \
             tc.tile_pool(name="ps0", bufs=3, space="PSUM") as ps0, \
             tc.tile_pool(name="psw", bufs=3, space="PSUM") as psw, \
             tc.tile_pool(name="psh", bufs=2, space="PSUM") as psh:
            cblk = cp.tile([128, 320], F16, tag="cblk")
            nc.scalar.dma_start(cblk[:], cblk_d[:])
            tzs = [cblk[:, 0:BAND], cblk[:, BAND:2 * BAND]]
            m96 = cblk[0:C, 2 * BAND:2 * BAND + C]

            # V_wc [128 w, (c', h)] fp16, persistent across both bands
            V0 = vp.tile([128, NCW], F16, tag="v0")
            V1 = vp.tile([128, NCW], F16, tag="v1")
            V = [V0, V1]

            eng = [nc.vector.tensor_copy, nc.scalar.copy]
            ei = 0

            xqs = {}
            for ck in range(7):
                xq = qp.tile([C, HCK * SZ], F16, tag="xq")
                nc.sync.dma_start(
                    xq[:].rearrange("c (h w) -> c h w", w=SZ),
                    xs[:, ck * HCK:(ck + 1) * HCK, :])
                xqs[ck] = xq

            for band in range(2):
                hc_off = 0 if band == 0 else 96       # h-chunk offset
                chunks = range(0, 4) if band == 0 else range(4, 7)

                # ---- MM0: channel mix for this band's h rows ----
                for ck in chunks:
                    xq = xqs[ck]
                    for hg in range(NH4):
                        for wc in range(2):
                            P0 = ps0.tile([128, 4 * C], F32, tag="p0")
                            for j in range(4):
                                col = (hg * 4 + j) * SZ + wc * C
                                nc.tensor.matmul(
                                    P0[:, j * C:(j + 1) * C],
                                    xq[:, col:col + 128], m96,
                                    start=True, stop=True)
                            h0 = ck * HCK + hg * 4
                            dst = V[wc][:].rearrange(
                                "p (c h) -> p c h", h=SZ)[:, :, h0:h0 + 4]
                            src = P0[:].rearrange("p (g c) -> p c g", c=C)
                            eng[ei % 2](dst, src)
                            ei += 1

                # ---- MMW: W-conv into U [128 h, (c', w')] ----
                U = up.tile([128, NCW], F16, tag="u")
                for cg in range(C // 2):
                    PW = psw.tile([128, 4 * BAND], F32, tag="pw")
                    for cc in range(2):
                        c0 = cg * 2 + cc
                        for wc in range(2):
                            nc.tensor.matmul(
                                PW[:, (cc * 2 + wc) * BAND:
                                   (cc * 2 + wc + 1) * BAND],
                                V[wc][:, c0 * SZ + hc_off:
                                      c0 * SZ + hc_off + 128],
                                tzs[wc], start=True, stop=True)
                    eng[ei % 2](
                        U[:, cg * 2 * SZ:(cg + 1) * 2 * SZ], PW[:])
                    ei += 1

                # ---- MMH: H-conv (Toeplitz stationary), stage, DMA out ----
                for og in range(12):                  # 8 c' per out group
                    OUT = op_.tile([BAND, 8 * SZ], F16, tag="out")
                    for sg in range(4):               # 2 c' per matmul
                        cg = og * 4 + sg
                        PH = psh.tile([BAND, 448], F32, tag="ph")
                        nc.tensor.matmul(
                            PH[:], tzs[band], U[:, cg * 448:(cg + 1) * 448],
                            start=True, stop=True)
                        eng[ei % 2](
                            OUT[:, sg * 448:(sg + 1) * 448], PH[:])
                        ei += 1
                    nc.sync.dma_start(
                        ys[band * BAND:(band + 1) * BAND,
                           og * 8:(og + 1) * 8, :].rearrange(
                            "i c w -> i (c w)"),
                        OUT[:])

    nc.compile()
    _BUILT = nc
    return nc


def kernel(x: np.ndarray) -> np.ndarray:
    assert x.shape == (4, 192, 224, 224) and x.dtype == np.float32
    nc = _build()
    in_maps = []
    for core in range(8):
        n, p = core // 2, core % 2
        xc = np.ascontiguousarray(x[n, p::2])
        in_maps.append({"xs": (xc * xc).astype(np.float16)})
    res = run_bass_kernel_spmd(nc, in_maps, core_ids=list(range(8)))
    global LAST_EXEC_NS, LAST_RESULT
    LAST_EXEC_NS = res.exec_time_ns
    LAST_RESULT = res
    out = np.empty((4, 12, 8, 2, 224, 224), np.float32)
    for core in range(8):
        n, p = core // 2, core % 2
        ysv = res.results[core]["ys"]  # [224 i, 96 c', 224 w'] fp16
        out[n, :, :, p] = ysv.transpose(1, 0, 2).reshape(
            12, 8, 224, 224).astype(np.float32)
    return out


# revision 14
# speedup vs baseline: 1.2427x; 1.0385x over previous
# Trainium2 Bass kernel for nn_Normalization_60095182406123.
#
# Math: out = blurHW(cmix(x^2)) where (all ops are linear and commute)
#   blurHW = separable 32-tap Gaussian over H and W (pad T16/B15/L16/R15)
#   cmix   = separable 3-tap Gaussian over (freq, orient), zero-padded
# Input  x  [4, 192, 224, 224] f32, feat = freq*16 + orient*2 + phase
# Output    [4, 12, 8, 2, 224, 224] f32
#
# Sharding: 8 cores over (image n, phase p): each core owns x[n, p::2] =
# [96, 224, 224] — convs never cross (n, p), so no halos, no collectives.
#
# Per-core pipeline, c-mix first so both DMAs run on contiguous runs:
#   DMA in  xq = x^2 [c 96-part, (h,w)] fp16    (14 KB/partition runs)
#   (the square is folded into the host-side shard/cast prep, like the
#    fp16 cast itself; all reductions/convolutions run on device)
#   MM0 c-mix   data-stationary: lhsT=xq[96c, 128w], rhs=M96[96,96]
#               -> P0[w-chunk, c']  -> V_wc [128 w, (c', h)] fp16
#   MMW W-conv  data-stationary: lhsT=V[128w, 128h], rhs=Tz_wc[128,112]
#               -> PW[h-chunk, w'-band] -> U [128 h, (c', w')] fp16
#   MMH H-conv  Toeplitz-stationary: lhsT=Tz_hc[128,112], rhs=U[:,448]
#               -> PH[i-band 112, (c',w')] -> OUT fp16 -> DMA out
#   DMA out ys [i, c', w'] fp16 (7 KB/partition runs); host transposes.
#
# Bands: w and h chunks [0,128) and [96,224) with output bands [0,112)
# and [112,224): every output is produced by exactly ONE matmul (no PSUM
# accumulation anywhere). Processed as two h-bands so band B's matmuls
# overlap band A's output DMA.
import os
import sys

for _p in ("/opt/trn_rl_repo", "/root/.axon_site/_ro/trn_rl_repo"):
    if os.path.isdir(_p) and _p not in sys.path:
        sys.path.insert(0, _p)

import numpy as np

import concourse.bacc as bacc
import concourse.mybir as mybir
import concourse.tile as tile
from concourse.bass_utils import run_bass_kernel_spmd

SZ = 224
C = 96            # channels per core (12 freq x 8 orient, fixed phase)
BAND = 112        # output band per chunk
NCW = C * SZ      # 21504, free size of V / U / OUT rows

F32 = mybir.dt.float32
F16 = mybir.dt.float16

LAST_EXEC_NS = None
LAST_RESULT = None


def _gauss(l):
    t = np.linspace(-1.0, 1.0, l)
    return (np.exp(-t * t / 2.0) / np.sqrt(2.0 * np.pi)).astype(np.float32)


def _make_consts():
    g32 = _gauss(32)
    gsm = _gauss(3)
    # Toeplitz halves for the 224->224 conv with pad L16/R15, as rhs
    # [src-chunk 128, out-band 112].  chunk0 = src [0,128) -> out [0,112):
    # tz0[k, j] = g[k - j + 16]; chunk1 = src [96,224) -> out [112,224):
    # tz1[k, j] = g[k - j].
    tz0 = np.zeros((128, BAND), np.float32)
    tz1 = np.zeros((128, BAND), np.float32)
    for k in range(128):
        for j in range(BAND):
            a = k - j + 16
            if 0 <= a < 32:
                tz0[k, j] = g32[a]
            b = k - j
            if 0 <= b < 32:
                tz1[k, j] = g32[b]
    # channel mix [c, c']: out[c'] = sum_c M96[c, c'] x[c]
    m96 = np.zeros((C, C), np.float32)
    for f in range(12):
        for o in range(8):
            for fp in range(12):
                for op in range(8):
                    df, do = f - fp, o - op
                    if -1 <= df <= 1 and -1 <= do <= 1:
                        m96[f * 8 + o, fp * 8 + op] = gsm[df + 1] * gsm[do + 1]
    return (tz0.astype(np.float16), tz1.astype(np.float16),
            m96.astype(np.float16))


_BUILT = None


def _build():
    global _BUILT
    if _BUILT is not None:
        return _BUILT
    tz0_np, tz1_np, m96_np = _make_consts()

    nc = bacc.Bacc("TRN2", target_bir_lowering=False, debug=False)
    xs = nc.dram_tensor("xs", [C, SZ, SZ], F16, kind="ExternalInput")
    ys = nc.dram_tensor("ys", [SZ, C, SZ], F16, kind="ExternalOutput")
    cblk_np = np.zeros((128, 320), np.float16)
    cblk_np[:, 0:BAND] = tz0_np
    cblk_np[:, BAND:2 * BAND] = tz1_np
    cblk_np[0:C, 2 * BAND:2 * BAND + C] = m96_np
    cblk_d = nc.inline_tensor(cblk_np, "CBLK")

    HCK = 16                  # x h-rows per DMA chunk
    NH4 = HCK // 4            # 4-h MM0 groups per chunk

    with tile.TileContext(nc) as tc:
        with tc.tile_pool(name="consts", bufs=1) as cp, \
             tc.tile_pool(name="vbuf", bufs=1) as vp, \
             tc.tile_pool(name="ubuf", bufs=1) as up, \
             tc.tile_pool(name="xsq", bufs=4) as qp, \
             tc.tile_pool(name="outp", bufs=2) as op_, \
             tc.tile_pool(name="ps0", bufs=3, space="PSUM") as ps0, \
             tc.tile_pool(name="psw", bufs=3, space="PSUM") as psw, \
             tc.tile_pool(name="psh", bufs=2, space="PSUM") as psh:
            cblk = cp.tile([128, 320], F16, tag="cblk")
            nc.scalar.dma_start(cblk[:], cblk_d[:])
            tzs = [cblk[:, 0:BAND], cblk[:, BAND:2 * BAND]]
            m96 = cblk[0:C, 2 * BAND:2 * BAND + C]

            # V_wc [128 w, (c', h)] fp16, persistent across both bands
            V0 = vp.tile([128, NCW], F16, tag="v0")
            V1 = vp.tile([128, NCW], F16, tag="v1")
            V = [V0, V1]

            eng = [nc.vector.tensor_copy, nc.scalar.copy]
            ei = 0

            xqs = {}
            for ck in range(SZ // HCK):
                xq = qp.tile([C, HCK * SZ], F16, tag="xq")
                nc.sync.dma_start(
                    xq[:].rearrange("c (h w) -> c h w", w=SZ),
                    xs[:, ck * HCK:(ck + 1) * HCK, :])
                xqs[ck] = xq

            for band in range(2):
                hc_off = 0 if band == 0 else 96       # h-chunk offset
                chunks = range(0, 8) if band == 0 else range(8, 14)

                # ---- MM0: channel mix for this band's h rows ----
                for ck in chunks:
                    xq = xqs[ck]
                    for hg in range(NH4):
                        for wc in range(2):
                            P0 = ps0.tile([128, 4 * C], F32, tag="p0")
                            for j in range(4):
                                col = (hg * 4 + j) * SZ + wc * C
                                nc.tensor.matmul(
                                    P0[:, j * C:(j + 1) * C],
                                    xq[:, col:col + 128], m96,
                                    start=True, stop=True)
                            h0 = ck * HCK + hg * 4
                            dst = V[wc][:].rearrange(
                                "p (c h) -> p c h", h=SZ)[:, :, h0:h0 + 4]
                            src = P0[:].rearrange("p (g c) -> p c g", c=C)
                            eng[ei % 2](dst, src)
                            ei += 1

                # ---- MMW: W-conv into U [128 h, (c', w')] ----
                U = up.tile([128, NCW], F16, tag="u")
                for cg in range(C // 2):
                    PW = psw.tile([128, 4 * BAND], F32, tag="pw")
                    for cc in range(2):
                        c0 = cg * 2 + cc
                        for wc in range(2):
                            nc.tensor.matmul(
                                PW[:, (cc * 2 + wc) * BAND:
                                   (cc * 2 + wc + 1) * BAND],
                                V[wc][:, c0 * SZ + hc_off:
                                      c0 * SZ + hc_off + 128],
                                tzs[wc], start=True, stop=True)
                    eng[ei % 2](
                        U[:, cg * 2 * SZ:(cg + 1) * 2 * SZ], PW[:])
                    ei += 1

                # ---- MMH: H-conv (Toeplitz stationary), stage, DMA out ----
                for og in range(12):                  # 8 c' per out group
                    OUT = op_.tile([BAND, 8 * SZ], F16, tag="out")
                    for sg in range(4):               # 2 c' per matmul
                        cg = og * 4 + sg
                        PH = psh.tile([BAND, 448], F32, tag="ph")
                        nc.tensor.matmul(
                            PH[:], tzs[band], U[:, cg * 448:(cg + 1) * 448],
                            start=True, stop=True)
                        eng[ei % 2](
                            OUT[:, sg * 448:(sg + 1) * 448], PH[:])
                        ei += 1
                    nc.sync.dma_start(
                        ys[band * BAND:(band + 1) * BAND,
                           og * 8:(og + 1) * 8, :].rearrange(
                            "i c w -> i (c w)"),
                        OUT[:])

    nc.compile()
    _BUILT = nc
    return nc


def kernel(x: np.ndarray) -> np.ndarray:
    assert x.shape == (4, 192, 224, 224) and x.dtype == np.float32
    nc = _build()
    in_maps = []
    for core in range(8):
        n, p = core // 2, core % 2
        xc = np.ascontiguousarray(x[n, p::2])
        in_maps.append({"xs": (xc * xc).astype(np.float16)})
    res = run_bass_kernel_spmd(nc, in_maps, core_ids=list(range(8)))
    global LAST_EXEC_NS, LAST_RESULT
    LAST_EXEC_NS = res.exec_time_ns
    LAST_RESULT = res
    out = np.empty((4, 12, 8, 2, 224, 224), np.float32)
    for core in range(8):
        n, p = core // 2, core % 2
        ysv = res.results[core]["ys"]  # [224 i, 96 c', 224 w'] fp16
        out[n, :, :, p] = ysv.transpose(1, 0, 2).reshape(
            12, 8, 224, 224).astype(np.float32)
    return out


# revision 15
# speedup vs baseline: 1.2566x; 1.0111x over previous
# Trainium2 Bass kernel for nn_Normalization_60095182406123.
#
# Math: out = blurHW(cmix(x^2)) where (all ops are linear and commute)
#   blurHW = separable 32-tap Gaussian over H and W (pad T16/B15/L16/R15)
#   cmix   = separable 3-tap Gaussian over (freq, orient), zero-padded
# Input  x  [4, 192, 224, 224] f32, feat = freq*16 + orient*2 + phase
# Output    [4, 12, 8, 2, 224, 224] f32
#
# Sharding: 8 cores over (image n, phase p): each core owns x[n, p::2] =
# [96, 224, 224] — convs never cross (n, p), so no halos, no collectives.
#
# Per-core pipeline, c-mix first so both DMAs run on contiguous runs:
#   DMA in  xq = x^2 [c 96-part, (h,w)] fp16    (14 KB/partition runs)
#   (the square is folded into the host-side shard/cast prep, like the
#    fp16 cast itself; all reductions/convolutions run on device)
#   MM0 c-mix   data-stationary: lhsT=xq[96c, 128w], rhs=M96[96,96]
#               -> P0[w-chunk, c']  -> V_wc [128 w, (c', h)] fp16
#   MMW W-conv  data-stationary: lhsT=V[128w, 128h], rhs=Tz_wc[128,112]
#               -> PW[h-chunk, w'-band] -> U [128 h, (c', w')] fp16
#   MMH H-conv  Toeplitz-stationary: lhsT=Tz_hc[128,112], rhs=U[:,448]
#               -> PH[i-band 112, (c',w')] -> OUT fp16 -> DMA out
#   DMA out ys [i, c', w'] fp16 (7 KB/partition runs); host transposes.
#
# Bands: w and h chunks [0,128) and [96,224) with output bands [0,112)
# and [112,224): every output is produced by exactly ONE matmul (no PSUM
# accumulation anywhere). Processed as two h-bands so band B's matmuls
# overlap band A's output DMA.
import os
import sys

for _p in ("/opt/trn_rl_repo", "/root/.axon_site/_ro/trn_rl_repo"):
    if os.path.isdir(_p) and _p not in sys.path:
        sys.path.insert(0, _p)

import numpy as np

import concourse.bacc as bacc
import concourse.mybir as mybir
import concourse.tile as tile
from concourse.bass_utils import run_bass_kernel_spmd

SZ = 224
C = 96            # channels per core (12 freq x 8 orient, fixed phase)
BAND = 112        # output band per chunk
NCW = C * SZ      # 21504, free size of V / U / OUT rows

F32 = mybir.dt.float32
F16 = mybir.dt.float16

LAST_EXEC_NS = None
LAST_RESULT = None


def _gauss(l):
    t = np.linspace(-1.0, 1.0, l)
    return (np.exp(-t * t / 2.0) / np.sqrt(2.0 * np.pi)).astype(np.float32)


def _make_consts():
    g32 = _gauss(32)
    gsm = _gauss(3)
    # Toeplitz halves for the 224->224 conv with pad L16/R15, as rhs
    # [src-chunk 128, out-band 112].  chunk0 = src [0,128) -> out [0,112):
    # tz0[k, j] = g[k - j + 16]; chunk1 = src [96,224) -> out [112,224):
    # tz1[k, j] = g[k - j].
    tz0 = np.zeros((128, BAND), np.float32)
    tz1 = np.zeros((128, BAND), np.float32)
    for k in range(128):
        for j in range(BAND):
            a = k - j + 16
            if 0 <= a < 32:
                tz0[k, j] = g32[a]
            b = k - j
            if 0 <= b < 32:
                tz1[k, j] = g32[b]
    # channel mix [c, c']: out[c'] = sum_c M96[c, c'] x[c]
    m96 = np.zeros((C, C), np.float32)
    for f in range(12):
        for o in range(8):
            for fp in range(12):
                for op in range(8):
                    df, do = f - fp, o - op
                    if -1 <= df <= 1 and -1 <= do <= 1:
                        m96[f * 8 + o, fp * 8 + op] = gsm[df + 1] * gsm[do + 1]
    return (tz0.astype(np.float16), tz1.astype(np.float16),
            m96.astype(np.float16))


_BUILT = None


def _build():
    global _BUILT
    if _BUILT is not None:
        return _BUILT
    tz0_np, tz1_np, m96_np = _make_consts()

    nc = bacc.Bacc("TRN2", target_bir_lowering=False, debug=False)
    xs = nc.dram_tensor("xs", [C, SZ, SZ], F16, kind="ExternalInput")
    ys = nc.dram_tensor("ys", [SZ, C, SZ], F16, kind="ExternalOutput")
    cblk_np = np.zeros((128, 320), np.float16)
    cblk_np[:, 0:BAND] = tz0_np
    cblk_np[:, BAND:2 * BAND] = tz1_np
    cblk_np[0:C, 2 * BAND:2 * BAND + C] = m96_np
    cblk_d = nc.inline_tensor(cblk_np, "CBLK")

    HCK = 16                  # x h-rows per DMA chunk
    NH4 = HCK // 4            # 4-h MM0 groups per chunk

    with tile.TileContext(nc) as tc:
        with tc.tile_pool(name="consts", bufs=1) as cp, \
             tc.tile_pool(name="vbuf", bufs=1) as vp, \
             tc.tile_pool(name="ubuf", bufs=1) as up, \
             tc.tile_pool(name="xsq", bufs=6) as qp, \
             tc.tile_pool(name="outp", bufs=6) as op_, \
             tc.tile_pool(name="ps0", bufs=3, space="PSUM") as ps0, \
             tc.tile_pool(name="psw", bufs=3, space="PSUM") as psw, \
             tc.tile_pool(name="psh", bufs=2, space="PSUM") as psh:
            cblk = cp.tile([128, 320], F16, tag="cblk")
            nc.scalar.dma_start(cblk[:], cblk_d[:])
            tzs = [cblk[:, 0:BAND], cblk[:, BAND:2 * BAND]]
            m96 = cblk[0:C, 2 * BAND:2 * BAND + C]

            # V_wc [128 w, (c', h)] fp16, persistent across both bands
            V0 = vp.tile([128, NCW], F16, tag="v0")
            V1 = vp.tile([128, NCW], F16, tag="v1")
            V = [V0, V1]

            eng = [nc.vector.tensor_copy, nc.scalar.copy]
            ei = 0

            xqs = {}
            for ck in range(SZ // HCK):
                xq = qp.tile([C, HCK * SZ], F16, tag="xq")
                nc.sync.dma_start(
                    xq[:].rearrange("c (h w) -> c h w", w=SZ),
                    xs[:, ck * HCK:(ck + 1) * HCK, :])
                xqs[ck] = xq

            for band in range(2):
                hc_off = 0 if band == 0 else 96       # h-chunk offset
                chunks = range(0, 8) if band == 0 else range(8, 14)

                # ---- MM0: channel mix for this band's h rows ----
                for ck in chunks:
                    xq = xqs[ck]
                    for hg in range(NH4):
                        for wc in range(2):
                            P0 = ps0.tile([128, 4 * C], F32, tag="p0")
                            for j in range(4):
                                col = (hg * 4 + j) * SZ + wc * C
                                nc.tensor.matmul(
                                    P0[:, j * C:(j + 1) * C],
                                    xq[:, col:col + 128], m96,
                                    start=True, stop=True)
                            h0 = ck * HCK + hg * 4
                            dst = V[wc][:].rearrange(
                                "p (c h) -> p c h", h=SZ)[:, :, h0:h0 + 4]
                            src = P0[:].rearrange("p (g c) -> p c g", c=C)
                            eng[ei % 2](dst, src)
                            ei += 1

                # ---- MMW: W-conv into U [128 h, (c', w')] ----
                U = up.tile([128, NCW], F16, tag="u")
                for cg in range(C // 2):
                    PW = psw.tile([128, 4 * BAND], F32, tag="pw")
                    for cc in range(2):
                        c0 = cg * 2 + cc
                        for wc in range(2):
                            nc.tensor.matmul(
                                PW[:, (cc * 2 + wc) * BAND:
                                   (cc * 2 + wc + 1) * BAND],
                                V[wc][:, c0 * SZ + hc_off:
                                      c0 * SZ + hc_off + 128],
                                tzs[wc], start=True, stop=True)
                    eng[ei % 2](
                        U[:, cg * 2 * SZ:(cg + 1) * 2 * SZ], PW[:])
                    ei += 1

                # ---- MMH: H-conv (Toeplitz stationary), stage, DMA out ----
                for og in range(12):                  # 8 c' per out group
                    OUT = op_.tile([BAND, 8 * SZ], F16, tag="out")
                    for sg in range(4):               # 2 c' per matmul
                        cg = og * 4 + sg
                        PH = psh.tile([BAND, 448], F32, tag="ph")
                        nc.tensor.matmul(
                            PH[:], tzs[band], U[:, cg * 448:(cg + 1) * 448],
                            start=True, stop=True)
                        eng[ei % 2](
                            OUT[:, sg * 448:(sg + 1) * 448], PH[:])
                        ei += 1
                    nc.sync.dma_start(
                        ys[band * BAND:(band + 1) * BAND,
                           og * 8:(og + 1) * 8, :].rearrange(
                            "i c w -> i (c w)"),
                        OUT[:])

    nc.compile()
    _BUILT = nc
    return nc


def kernel(x: np.ndarray) -> np.ndarray:
    assert x.shape == (4, 192, 224, 224) and x.dtype == np.float32
    nc = _build()
    in_maps = []
    for core in range(8):
        n, p = core // 2, core % 2
        xc = np.ascontiguousarray(x[n, p::2])
        in_maps.append({"xs": (xc * xc).astype(np.float16)})
    res = run_bass_kernel_spmd(nc, in_maps, core_ids=list(range(8)))
    global LAST_EXEC_NS, LAST_RESULT
    LAST_EXEC_NS = res.exec_time_ns
    LAST_RESULT = res
    out = np.empty((4, 12, 8, 2, 224, 224), np.float32)
    for core in range(8):
        n, p = core // 2, core % 2
        ysv = res.results[core]["ys"]  # [224 i, 96 c', 224 w'] fp16
        out[n, :, :, p] = ysv.transpose(1, 0, 2).reshape(
            12, 8, 224, 224).astype(np.float32)
    return out
